# revision 5
# baseline (speedup 1.0000x reference)
"""Trainium2 Bass kernel for nn_CARFACCell.

Math: per-row linear recurrence a[t] = f[t]*a[t-1] + g[t] over T=4096 (init a0),
followed by `steps` iterations of a symmetric-padded valid 5-tap cross-correlation
along T.  Rows = B*C = 4096, sharded 512 rows per core across 8 cores (core b
takes batch b).

Per-core plan:
  - 4 row-tiles of [128 rows, 4096 t] in SBUF.
  - Recurrence: one native DVE tensor_tensor_scan per row-tile (scan along free
    dim, initial = a0 column).
  - Smoothing: the composed `steps`-fold operator is a banded matrix (17-tap
    interior, boundary-folded first/last 8 outputs).  For each 112-wide output
    window: PE-transpose the [128,128] t-input block of rec, copy PSUM->SBUF,
    then one fp32 matmul  out = X_T.T @ W  (X_T stationary, W moving) giving a
    [128 rows, 112 t_out] block in row layout.  W windows are built numerically
    on the host from the runtime `kernel` taps (exact, incl. boundary folds).
"""

import os
import numpy as np

import concourse.bacc as bacc
import concourse.tile as tile
from concourse import mybir
from concourse.bass_utils import run_bass_kernel_spmd

B, C, T = 8, 512, 4096
N_CORES = 8
ROWS = B * C // N_CORES      # 512 rows per core
NRT = ROWS // 128            # 4 row-tiles per core
HALO = 8                     # steps * (K-1)//2 for steps=4, K=5
WIN = 128 - 2 * HALO         # 112 output columns per conv window
NW = (T + WIN - 1) // WIN    # 37 windows (last one overlaps)
F32 = mybir.dt.float32

_NC_CACHE = {}


def _build_w_windows(k5: np.ndarray, steps: int):
    """Build the three distinct [128, WIN] fp32 weight windows of the composed
    smoothing operator, numerically exact (including symmetric-pad boundary
    folds).  M[s, t] = d out[t] / d in[s]."""
    K = len(k5)
    pad = (K - 1) // 2
    assert steps * pad == HALO, (steps, K)
    L = 512
    M = np.eye(L, dtype=np.float64)
    k5 = np.asarray(k5, dtype=np.float64)
    for _ in range(steps):
        Mp = np.pad(M, ((0, 0), (pad, pad)), mode="symmetric")
        M = sum(k5[i] * Mp[:, i:i + L] for i in range(K))
    # interior window at j0=2 (boundary-free for L=512)
    w_first = M[0:128, 0:WIN]
    w_mid = M[2 * WIN - HALO: 2 * WIN - HALO + 128, 2 * WIN: 3 * WIN]
    w_last = M[L - 128: L, L - WIN: L]
    return (np.ascontiguousarray(w_first, dtype=np.float32),
            np.ascontiguousarray(w_mid, dtype=np.float32),
            np.ascontiguousarray(w_last, dtype=np.float32))


def _window_geometry():
    """Yield (t_in_start, t_out_start, out_col_start, psum_col_start, ncols, which_w)."""
    geo = []
    for j in range(NW):
        if j == 0:
            geo.append((0, 0, 0, 0, WIN, 0))
        elif j == NW - 1:
            # last window right-aligned; copy only the non-overlapping tail
            to = T - WIN
            prev_end = WIN * (NW - 1)
            tail = T - prev_end                       # 64
            geo.append((T - 128, to, prev_end, WIN - tail, tail, 2))
        else:
            geo.append((WIN * j - HALO, WIN * j, WIN * j, 0, WIN, 1))
    return geo


def _build_nc():
    nc = bacc.Bacc("TRN2", target_bir_lowering=False, debug=False)

    f_d = nc.dram_tensor("f", [ROWS, T], F32, kind="ExternalInput").ap()
    g_d = nc.dram_tensor("g", [ROWS, T], F32, kind="ExternalInput").ap()
    a0_d = nc.dram_tensor("a0", [ROWS, 1], F32, kind="ExternalInput").ap()
    wf_d = nc.dram_tensor("w_first", [128, WIN], F32, kind="ExternalInput").ap()
    wm_d = nc.dram_tensor("w_mid", [128, WIN], F32, kind="ExternalInput").ap()
    wl_d = nc.dram_tensor("w_last", [128, WIN], F32, kind="ExternalInput").ap()
    id_d = nc.dram_tensor("ident", [128, 128], F32, kind="ExternalInput").ap()
    out_d = nc.dram_tensor("out", [ROWS, T], F32, kind="ExternalOutput").ap()

    geo = _window_geometry()

    with tile.TileContext(nc) as tc:
        with (
            tc.tile_pool(name="const", bufs=1) as const_pool,
            tc.tile_pool(name="fg", bufs=2) as fg_pool,
            tc.tile_pool(name="rec", bufs=2) as rec_pool,
            tc.tile_pool(name="outp", bufs=2) as out_pool,
            tc.tile_pool(name="xts", bufs=4) as xts_pool,
            tc.tile_pool(name="xtp", bufs=3, space="PSUM") as xtp_pool,
            tc.tile_pool(name="cvp", bufs=3, space="PSUM") as cvp_pool,
        ):
            idt = const_pool.tile([128, 128], F32)
            nc.sync.dma_start(idt[:], id_d)
            w_tiles = []
            for nm, d in (("wf", wf_d), ("wm", wm_d), ("wl", wl_d)):
                wt = const_pool.tile([128, WIN], F32, tag=nm, name=nm)
                nc.sync.dma_start(wt[:], d)
                w_tiles.append(wt)

            LCH = T // 2   # load chunk columns
            SCH = T // 4   # scan chunk columns
            OCH = T // 4   # output store chunk columns
            for rt in range(NRT):
                r0 = rt * 128
                f_t = fg_pool.tile([128, T], F32, tag="f", name="f_t")
                g_t = fg_pool.tile([128, T], F32, tag="g", name="g_t")
                for c0 in range(0, T, LCH):
                    nc.sync.dma_start(f_t[:, c0:c0 + LCH],
                                      f_d[r0:r0 + 128, c0:c0 + LCH])
                    nc.sync.dma_start(g_t[:, c0:c0 + LCH],
                                      g_d[r0:r0 + 128, c0:c0 + LCH])
                a0_t = fg_pool.tile([128, 1], F32, tag="a0", name="a0_t")
                nc.sync.dma_start(a0_t[:], a0_d[r0:r0 + 128, :])

                # chunked scan chained through `initial` so the conv below can
                # start as soon as the first chunk of rec exists
                rec_t = rec_pool.tile([128, T], F32, name="rec_t")
                for c0 in range(0, T, SCH):
                    init = a0_t[:, 0:1] if c0 == 0 else rec_t[:, c0 - 1:c0]
                    nc.vector.tensor_tensor_scan(
                        rec_t[:, c0:c0 + SCH], f_t[:, c0:c0 + SCH],
                        g_t[:, c0:c0 + SCH], initial=init,
                        op0=mybir.AluOpType.mult, op1=mybir.AluOpType.add,
                    )

                out_t = out_pool.tile([128, T], F32, name="out_t")
                # groups of up to 4 windows share one PSUM bank on each side
                # so the PSUM->SBUF copies are wide (amortize fixed costs)
                for gi, g0 in enumerate(range(0, NW, 4)):
                    wins = geo[g0:g0 + 4]
                    nwin = len(wins)
                    xtp = xtp_pool.tile([128, 128 * nwin], F32, name="xtp",
                                        tag="xtp")
                    for k, (ti, *_rest) in enumerate(wins):
                        nc.tensor.transpose(xtp[:, 128 * k:128 * (k + 1)],
                                            rec_t[:, ti:ti + 128], idt[:])
                    xts = xts_pool.tile([128, 128 * nwin], F32, name="xts",
                                        tag="xts")
                    nc.scalar.copy(xts[:], xtp[:])
                    cvp = cvp_pool.tile([128, WIN * nwin], F32, name="cvp",
                                        tag="cvp")
                    for k, (ti, to, oc, pc, ncols, wsel) in enumerate(wins):
                        nc.tensor.matmul(cvp[:, WIN * k:WIN * (k + 1)],
                                         lhsT=xts[:, 128 * k:128 * (k + 1)],
                                         rhs=w_tiles[wsel][:],
                                         start=True, stop=True)
                    # group's out columns are contiguous except the final
                    # (single-window, right-aligned) group
                    oc0 = wins[0][2]
                    pc0 = wins[0][3]
                    ncols_g = sum(w[4] for w in wins)
                    copy_eng = nc.vector if gi % 2 == 0 else nc.scalar
                    if nwin == 1:
                        src = cvp[:, pc0:pc0 + ncols_g]
                    else:
                        src = cvp[:, 0:ncols_g]
                    if copy_eng is nc.vector:
                        nc.vector.tensor_copy(out_t[:, oc0:oc0 + ncols_g], src)
                    else:
                        nc.scalar.copy(out_t[:, oc0:oc0 + ncols_g], src)
                for c0 in range(0, T, OCH):
                    nc.sync.dma_start(out_d[r0:r0 + 128, c0:c0 + OCH],
                                      out_t[:, c0:c0 + OCH])

    nc.compile()
    return nc


def kernel(a0, f, g, kernel, steps):
    a0 = np.ascontiguousarray(np.asarray(a0), dtype=np.float32)
    f = np.ascontiguousarray(np.asarray(f), dtype=np.float32)
    g = np.ascontiguousarray(np.asarray(g), dtype=np.float32)
    k5 = np.asarray(kernel, dtype=np.float64)
    steps = int(steps)

    assert f.shape == (B, C, T) and g.shape == (B, C, T) and a0.shape == (B, C)

    wf, wm, wl = _build_w_windows(k5, steps)
    ident = np.eye(128, dtype=np.float32)

    if "nc" not in _NC_CACHE:
        _NC_CACHE["nc"] = _build_nc()
    nc = _NC_CACHE["nc"]

    rows_per = ROWS  # 512; core b <- batch b (B == N_CORES)
    in_maps = []
    for b in range(N_CORES):
        in_maps.append({
            "f": f[b],
            "g": g[b],
            "a0": a0[b].reshape(ROWS, 1),
            "w_first": wf,
            "w_mid": wm,
            "w_last": wl,
            "ident": ident,
        })

    trace = os.environ.get("CARFAC_TRACE") == "1"
    if trace:
        try:
            import antenv.axon_hooks  # noqa: F401
        except ImportError:
            trace = False
    res = run_bass_kernel_spmd(nc, in_maps, list(range(N_CORES)), trace=trace)
    if trace and res.exec_time_ns is not None:
        print(f"HW exec time: {res.exec_time_ns} ns")
        _NC_CACHE["exec_time_ns"] = res.exec_time_ns

    out = np.stack([res.results[b]["out"] for b in range(N_CORES)], axis=0)
    return out.reshape(B, C, T)


def bench(a0, f, g, kernel, steps, iters=10):
    """Time the sharded PJRT executable with device-resident inputs.
    Returns (min_wall_s, all_times). Not used by grading."""
    import time
    import jax
    from jax.sharding import Mesh, PartitionSpec
    from jax.experimental.shard_map import shard_map
    from concourse import bass2jax, mybir as _mybir

    a0 = np.asarray(a0, np.float32)
    f = np.asarray(f, np.float32)
    g = np.asarray(g, np.float32)
    wf, wm, wl = _build_w_windows(np.asarray(kernel, np.float64), int(steps))
    ident = np.eye(128, dtype=np.float32)

    if "nc" not in _NC_CACHE:
        _NC_CACHE["nc"] = _build_nc()
    nc = _NC_CACHE["nc"]

    bass2jax.install_neuronx_cc_hook()
    partition_name = (nc.partition_id_tensor.name
                      if nc.partition_id_tensor else None)
    in_names, out_names, out_avals, zero_outs = [], [], [], []
    for alloc in nc.m.functions[0].allocations:
        if not isinstance(alloc, _mybir.MemoryLocationSet):
            continue
        name = alloc.memorylocations[0].name
        if alloc.kind == "ExternalInput":
            if name != partition_name:
                in_names.append(name)
        elif alloc.kind == "ExternalOutput":
            shape = tuple(alloc.tensor_shape)
            dtype = _mybir.dt.np(alloc.dtype)
            out_names.append(name)
            out_avals.append(jax.core.ShapedArray(shape, dtype))
            zero_outs.append(np.zeros(shape, dtype))
    n_params = len(in_names)
    n_outs = len(out_avals)
    all_in_names = list(in_names) + out_names
    if partition_name is not None:
        all_in_names.append(partition_name)

    def _body(*args):
        operands = list(args)
        if partition_name is not None:
            operands.append(bass2jax.partition_id_tensor())
        outs = bass2jax._bass_exec_p.bind(
            *operands,
            out_avals=tuple(out_avals),
            in_names=tuple(all_in_names),
            out_names=tuple(out_names),
            lowering_input_output_aliases=(),
            sim_require_finite=True,
            sim_require_nnan=True,
            nc=nc,
        )
        return tuple(outs)

    in_maps = []
    for b in range(N_CORES):
        in_maps.append({"f": f[b], "g": g[b], "a0": a0[b].reshape(ROWS, 1),
                        "w_first": wf, "w_mid": wm, "w_last": wl,
                        "ident": ident})
    per_core = [[np.asarray(m[nm]) for nm in in_names] for m in in_maps]
    concat_in = [np.concatenate([per_core[c][i] for c in range(N_CORES)], axis=0)
                 for i in range(n_params)]
    concat_zeros = [np.zeros((N_CORES * z.shape[0], *z.shape[1:]), z.dtype)
                    for z in zero_outs]

    devices = jax.devices()[:N_CORES]
    mesh = Mesh(np.asarray(devices), ("core",))
    sharded = jax.jit(
        shard_map(_body, mesh=mesh,
                  in_specs=(PartitionSpec("core"),) * (n_params + n_outs),
                  out_specs=(PartitionSpec("core"),) * n_outs,
                  check_rep=False),
        donate_argnums=tuple(range(n_params, n_params + n_outs)),
        keep_unused=True,
    )

    in_dev = [jax.device_put(x) for x in concat_in]
    times = []
    out = None
    for _ in range(iters):
        zeros_dev = [jax.device_put(z) for z in concat_zeros]
        jax.block_until_ready(zeros_dev)
        jax.block_until_ready(in_dev)
        t0 = time.perf_counter()
        out = sharded(*in_dev, *zeros_dev)
        jax.block_until_ready(out)
        times.append(time.perf_counter() - t0)
    return min(times), times, out
